# revision 13
# baseline (speedup 1.0000x reference)
"""GCN (single GCNConv + Cox head) Trainium2 Bass kernel, 8-core SPMD.

Math (per reference):
    src,dst += self loops;  deg = indegree(dst);  dinv = deg^-1/2
    agg[d]  = dinv[d] * sum_e 1[dst_e = d] * (dinv[src_e] * x[src_e])
    out     = relu(agg @ W.T + b) @ w_reg.T + b_reg

Distribution: destination-sharded over 8 cores (12500 dst nodes each), no
collectives — each core gets its own relabeled tables + scatter matrices
and writes its output shard; the host concatenates shards.

Per core the dst range is cut into 128-node blocks; blocks into groups of
GRP=4 (one [128, 512] PSUM bank per group). Per block, slots = the block's
DISTINCT sources (dinv[src] pre-folded into the stored fp16 rows); each
slot's scatter row carries the edge multiplicity at every dst it feeds
(counts are exact in fp8), so repeated sources need no extra slots and no
gather path at all. Slots are padded per block to NB_k*128 (NB_k shared
across cores so the SPMD instruction stream is identical).

Pipeline per group (all HWDGE-streamed, no SWDGE):
  - stream DMA: slot rows in matmul layout (slot i -> partition i%128)
  - scatter DMA: oh[slot, dst] fp8, one 128-col batch per 128 slots
  - PE: psum[f, dst] += msg[slot, f].T @ oh[slot, dst]  (fp16 x fp8)
  - DVE: accq = psum * dinv_dst (per-column, via gpsimd-broadcast row)
  - PE/ACT: h = relu(W.T @ accq + b); cox row = w_reg.T @ h + b_reg
  - one DMA writes the [1, 12544] output row at the end.
"""

import os
import time
import numpy as np

N_CORES = 8
BLK = 128      # dst nodes per block == scatter window
GRP = 4        # blocks per group == one [128, 512] psum bank
CH = GRP * BLK


class Plan:
    def __init__(self, n_feat, nblk, nb_of_blk):
        self.F = n_feat
        self.NBLK = nblk
        self.NB = nb_of_blk                      # batches per block [nblk]
        self.PREF = np.concatenate([[0], np.cumsum(nb_of_blk)])
        self.TOTB = int(self.PREF[-1])
        self.NPAD = nblk * BLK
        self.in_maps = []


def make_plan(x, edge_index, W, b, w_reg, b_reg, n_cores=N_CORES):
    x = np.asarray(x, dtype=np.float32)
    N, F = x.shape
    ns = N // n_cores
    assert ns * n_cores == N
    nblk = (ns + BLK - 1) // BLK

    src = np.asarray(edge_index[0], dtype=np.int64)
    dst = np.asarray(edge_index[1], dtype=np.int64)
    deg = (np.bincount(dst, minlength=N) + 1).astype(np.float64)
    dinv = 1.0 / np.sqrt(deg)
    xs = (x * dinv[:, None]).astype(np.float16)  # dinv_src folded into rows

    # per-core deduped (block, src) tables
    cores = []
    counts = []
    for c in range(n_cores):
        lo, hi = c * ns, (c + 1) * ns
        m = (dst >= lo) & (dst < hi)
        s_e = np.concatenate([src[m], np.arange(lo, hi)])
        d_e = np.concatenate([dst[m] - lo, np.arange(ns)])
        blk_e = d_e >> 7
        rel_e = (d_e & 127).astype(np.int64)
        uq, inv = np.unique(blk_e * N + s_e, return_inverse=True)
        ublk = uq // N
        usrc = uq % N
        c_k = np.bincount(ublk, minlength=nblk)
        slot_u = np.arange(len(uq)) - np.concatenate(
            [[0], np.cumsum(c_k)])[ublk]
        cores.append((blk_e, rel_e, inv, ublk, usrc, slot_u))
        counts.append(c_k)

    # rank alignment: each core processes its blocks sorted by slot count so
    # the SPMD-shared per-position batch count tracks the aligned quantiles,
    # not the worst core at each block id. perms[c][t] = core-c block at
    # position t; host un-permutes the output.
    perms = [np.argsort(-c_k, kind="stable") for c_k in counts]
    cnt_pos = np.stack([counts[c][perms[c]] for c in range(n_cores)])
    nb_of_blk = np.maximum(1, -(-cnt_pos.max(axis=0) // 128))  # per position
    plan = Plan(F, nblk, nb_of_blk)
    pref = plan.PREF

    import concourse.mybir as _mybir
    ohnp = _mybir.dt.np(_mybir.dt.float8e4)

    consts = {
        "wt": np.ascontiguousarray(
            np.asarray(W, np.float32).T).astype(np.float16),
        "bvec": np.asarray(b, np.float32).reshape(F, 1),
        "wreg": np.ascontiguousarray(
            np.asarray(w_reg, np.float32).T).astype(np.float16),
        "breg": np.asarray(b_reg, np.float32).reshape(1, 1),
    }

    # group tables (shared): runlen/rowbase per group of GRP blocks
    ngrp = -(-nblk // GRP)
    g_of_blk = np.arange(nblk) // GRP
    runlen = np.array([pref[min((g + 1) * GRP, nblk)] - pref[g * GRP]
                       for g in range(ngrp)])
    plan.RUNLEN = runlen

    plan.perms = perms
    for c in range(n_cores):
        blk_e, rel_e, inv, ublk, usrc, slot_u = cores[c]
        lo = c * ns
        posmap = np.empty(nblk, dtype=np.int64)
        posmap[perms[c]] = np.arange(nblk)

        # slot rows: row = 128*pref[g*GRP] + p*runlen_g + (pref[t]-pref[g*GRP]) + j
        upos = posmap[ublk]
        g_u = g_of_blk[upos]
        p_u = slot_u % 128
        j_u = slot_u // 128
        assert np.all(j_u < nb_of_blk[upos])
        row_u = (128 * pref[g_u * GRP] + p_u * runlen[g_u]
                 + (pref[upos] - pref[g_u * GRP]) + j_u)
        xg = np.zeros((128 * plan.TOTB, F), dtype=np.float16)
        xg[row_u] = xs[usrc]

        # scatter matrix with multiplicities
        slot_e = slot_u[inv]
        col_e = (pref[posmap[blk_e]] + slot_e // 128) * 128 + rel_e
        oh_u = np.zeros((128, plan.TOTB * 128), dtype=np.uint16)
        np.add.at(oh_u, (slot_e % 128, col_e), 1)
        oh = oh_u.astype(np.float32).astype(ohnp)

        dpad = np.ones(plan.NPAD, dtype=np.float16)
        dpad[:ns] = dinv[lo:lo + ns].astype(np.float16)
        dp = dpad.reshape(nblk, BLK)[perms[c]].reshape(1, plan.NPAD)

        plan.in_maps.append({
            "xg": xg,
            "oh": np.ascontiguousarray(oh),
            "dinvp": np.ascontiguousarray(dp),
            **consts,
        })
    return plan


# ---------------------------------------------------------------------------
def build_nc(plan):
    import concourse.bacc as bacc
    import concourse.mybir as mybir
    import concourse.tile as tile

    f32 = mybir.dt.float32
    f16 = mybir.dt.float16
    oh8 = mybir.dt.float8e4
    F, NBLK, NPAD, TOTB = plan.F, plan.NBLK, plan.NPAD, plan.TOTB
    NB, PREF, RUNLEN = plan.NB, plan.PREF, plan.RUNLEN
    NGRP = len(RUNLEN)
    RMAX = int(RUNLEN.max())

    nc = bacc.Bacc("TRN2", target_bir_lowering=False, debug=False)

    xg = nc.dram_tensor("xg", [128 * TOTB, F], f16, kind="ExternalInput").ap()
    oh = nc.dram_tensor("oh", [128, TOTB * 128], oh8,
                        kind="ExternalInput").ap()
    dinvp = nc.dram_tensor("dinvp", [1, NPAD], f16, kind="ExternalInput").ap()
    wt = nc.dram_tensor("wt", [F, F], f16, kind="ExternalInput").ap()
    bvec = nc.dram_tensor("bvec", [F, 1], f32, kind="ExternalInput").ap()
    wreg = nc.dram_tensor("wreg", [F, 1], f16, kind="ExternalInput").ap()
    breg = nc.dram_tensor("breg", [1, 1], f32, kind="ExternalInput").ap()
    out = nc.dram_tensor("out", [1, NPAD], f32, kind="ExternalOutput").ap()

    mult = mybir.AluOpType.mult
    bypass = mybir.AluOpType.bypass

    with tile.TileContext(nc) as tc:
        with (
            tc.tile_pool(name="const", bufs=1) as cpool,
            tc.tile_pool(name="stream", bufs=4) as spool,
            tc.tile_pool(name="ohp", bufs=4) as opool,
            tc.tile_pool(name="dbp", bufs=2) as dpool,
            tc.tile_pool(name="ps", bufs=3, space="PSUM") as pspool,
            tc.tile_pool(name="hq", bufs=3) as hqpool,
            tc.tile_pool(name="ph2", bufs=2, space="PSUM") as ph2pool,
            tc.tile_pool(name="po", bufs=2, space="PSUM") as popool,
            tc.tile_pool(name="hrelu", bufs=2) as hpool,
        ):
            wt_sb = cpool.tile([F, F], f16)
            b_sb = cpool.tile([F, 1], f32)
            wreg_sb = cpool.tile([F, 1], f16)
            breg_sb = cpool.tile([1, 1], f32)
            dinvp_sb = cpool.tile([1, NPAD], f16)
            out_sb = cpool.tile([1, NPAD], f32)

            def issue_group_dma(g):
                rl = int(RUNLEN[g])
                r0 = 128 * int(PREF[g * GRP])
                st = spool.tile([128, RMAX * F], f16, tag="st")
                nc.sync.dma_start(
                    out=st[:, :rl * F].rearrange("p (c f) -> p c f", f=F),
                    in_=xg[r0:r0 + 128 * rl, :].rearrange(
                        "(p c) f -> p c f", p=128),
                )
                ot = opool.tile([128, RMAX * BLK], oh8, tag="ot")
                nc.scalar.dma_start(
                    out=ot[:, :rl * BLK],
                    in_=oh[:, int(PREF[g * GRP]) * BLK:
                           (int(PREF[g * GRP]) + rl) * BLK])
                return st, ot

            # group 0's DMAs go first: descriptor-issue instructions cost
            # ~0.6-1.4us each on the issuing engine, so consts behind them
            # ride out group 0's transfer time instead of delaying it
            tiles0 = issue_group_dma(0)
            for sb, dr in ((dinvp_sb, dinvp), (wt_sb, wt), (b_sb, bvec),
                           (wreg_sb, wreg), (breg_sb, breg)):
                nc.sync.dma_start(out=sb[:], in_=dr[:])

            def phase2(hq, k0, cw):
                ph = ph2pool.tile([128, CH], f32)
                nc.tensor.matmul(ph[:, :cw], lhsT=wt_sb[:],
                                 rhs=hq[:, :cw], start=True, stop=True)
                hr = hpool.tile([128, CH], f16, tag="hr")
                nc.scalar.activation(hr[:, :cw], ph[:, :cw],
                                     mybir.ActivationFunctionType.Relu,
                                     bias=b_sb[:, :1])
                po = popool.tile([1, CH], f32)
                nc.tensor.matmul(po[:, :cw], lhsT=wreg_sb[:],
                                 rhs=hr[:, :cw], start=True, stop=True)
                nc.scalar.activation(out_sb[:, k0 * BLK:k0 * BLK + cw],
                                     po[:, :cw],
                                     mybir.ActivationFunctionType.Identity,
                                     bias=breg_sb[:, :1])
                # flush this group's output slice right away so only the
                # last slice remains for the kernel tail
                nc.scalar.dma_start(out=out[:, k0 * BLK:k0 * BLK + cw],
                                    in_=out_sb[:, k0 * BLK:k0 * BLK + cw])

            pending = None  # phase 2 lags one group so its matmuls never
            for g in range(NGRP):  # head-of-line block the PE FIFO
                k0 = g * GRP
                kn = min(GRP, NBLK - k0)
                cw = kn * BLK
                rl = int(RUNLEN[g])

                st, ot = tiles0 if g == 0 else issue_group_dma(g)

                db = dpool.tile([128, CH], f16, tag="db")
                nc.gpsimd.partition_broadcast(
                    db[:, :cw], dinvp_sb[:, k0 * BLK:k0 * BLK + cw])

                ps = pspool.tile([128, CH], f32)
                for bi in range(kn):
                    k = k0 + bi
                    nbk = int(NB[k])
                    base = int(PREF[k]) - int(PREF[k0])
                    for j in range(nbk):
                        cix = base + j
                        nc.tensor.matmul(
                            ps[:, bi * BLK:(bi + 1) * BLK],
                            lhsT=st[:, cix * F:(cix + 1) * F],
                            rhs=ot[:, cix * BLK:(cix + 1) * BLK],
                            start=(j == 0), stop=(j == nbk - 1))

                hq = hqpool.tile([128, CH], f16, tag="hq")
                nc.vector.scalar_tensor_tensor(
                    out=hq[:, :cw], in0=ps[:, :cw], scalar=1.0,
                    in1=db[:, :cw], op0=bypass, op1=mult)

                if pending is not None:
                    phase2(*pending)
                pending = (hq, k0, cw)
            phase2(*pending)

    nc.compile()
    return nc


# ---------------------------------------------------------------------------
_CACHE = {}


def _ensure_ntff_hook():
    try:
        from antenv.axon_hooks import get_axon_ntff_profile_hook  # noqa: F401
        return
    except ImportError:
        pass
    import sys
    import types
    import antenv
    mod = types.ModuleType("antenv.axon_hooks")
    mod._hook = None
    mod.set_axon_ntff_profile_hook = lambda h: setattr(mod, "_hook", h)
    mod.get_axon_ntff_profile_hook = lambda: mod._hook
    sys.modules["antenv.axon_hooks"] = mod
    antenv.axon_hooks = mod
    try:
        from trn_agent_boot.trn_boot import _ntff_profile_via_ctypes
        mod._hook = _ntff_profile_via_ctypes("/opt/axon/libaxon_pjrt.so")
    except Exception:
        pass


def _run(plan, nc, trace=False):
    import concourse.bass_utils as bu
    if trace:
        _ensure_ntff_hook()
        bu.upload_artifacts = lambda tmpdir: tmpdir  # no egress here
    core_ids = list(range(len(plan.in_maps)))
    res = bu.run_bass_kernel_spmd(nc, plan.in_maps, core_ids, trace=trace)
    return res


def kernel(x, edge_index, W, b, w_reg, b_reg):
    trace = bool(os.environ.get("GCN_TRACE"))

    plan = make_plan(x, edge_index, W, b, w_reg, b_reg)
    key = (plan.NBLK, plan.TOTB)
    if key not in _CACHE:
        _CACHE[key] = build_nc(plan)
    nc = _CACHE[key]

    res = None
    for attempt in range(3):
        try:
            res = _run(plan, nc, trace=trace)
            break
        except Exception:
            # transient device errors (e.g. NRT exec-unit resets) recover on
            # a fresh attempt; re-raise only if persistent
            if attempt == 2:
                raise
            time.sleep(5.0)
    kernel.last_exec_ns = res.exec_time_ns
    kernel.last_profile = res.profile_json

    N = np.asarray(x).shape[0]
    ns = N // len(plan.in_maps)
    shards = []
    for c in range(len(plan.in_maps)):
        o = res.results[c]["out"][0].reshape(plan.NBLK, BLK)
        unperm = np.empty_like(o)
        unperm[plan.perms[c]] = o  # position t holds block perms[c][t]
        shards.append(unperm.reshape(-1)[:ns])
    return np.concatenate(shards).reshape(N, 1).astype(np.float32)


kernel.last_exec_ns = None
kernel.last_profile = None


# revision 14
# speedup vs baseline: 1.0249x; 1.0249x over previous
"""GCN (single GCNConv + Cox head) Trainium2 Bass kernel, 8-core SPMD.

Math (per reference):
    src,dst += self loops;  deg = indegree(dst);  dinv = deg^-1/2
    agg[d]  = dinv[d] * sum_e 1[dst_e = d] * (dinv[src_e] * x[src_e])
    out     = relu(agg @ W.T + b) @ w_reg.T + b_reg

Distribution: destination-sharded over 8 cores (12500 dst nodes each), no
collectives — each core gets its own relabeled tables + scatter matrices
and writes its output shard; the host concatenates shards.

Per core the dst range is cut into 128-node blocks; blocks into groups of
GRP=4 (one [128, 512] PSUM bank per group). Per block, slots = the block's
DISTINCT sources (dinv[src] pre-folded into the stored fp16 rows); each
slot's scatter row carries the edge multiplicity at every dst it feeds
(counts are exact in fp8), so repeated sources need no extra slots and no
gather path at all. Slots are padded per block to NB_k*128 (NB_k shared
across cores so the SPMD instruction stream is identical).

Pipeline per group (all HWDGE-streamed, no SWDGE):
  - stream DMA: slot rows in matmul layout (slot i -> partition i%128)
  - scatter DMA: oh[slot, dst] fp8, one 128-col batch per 128 slots
  - PE: psum[f, dst] += msg[slot, f].T @ oh[slot, dst]  (fp16 x fp8)
  - DVE: accq = psum * dinv_dst (per-column, via gpsimd-broadcast row)
  - PE/ACT: h = relu(W.T @ accq + b); cox row = w_reg.T @ h + b_reg
  - one DMA writes the [1, 12544] output row at the end.
"""

import os
import time
import numpy as np

N_CORES = 8
BLK = 128      # dst nodes per block == scatter window
GRP = 4        # blocks per group == one [128, 512] psum bank
CH = GRP * BLK


class Plan:
    def __init__(self, n_feat, nblk, nb_of_blk):
        self.F = n_feat
        self.NBLK = nblk
        self.NB = nb_of_blk                      # batches per block [nblk]
        self.PREF = np.concatenate([[0], np.cumsum(nb_of_blk)])
        self.TOTB = int(self.PREF[-1])
        self.NPAD = nblk * BLK
        self.in_maps = []


def make_plan(x, edge_index, W, b, w_reg, b_reg, n_cores=N_CORES):
    x = np.asarray(x, dtype=np.float32)
    N, F = x.shape
    ns = N // n_cores
    assert ns * n_cores == N
    nblk = (ns + BLK - 1) // BLK

    src = np.asarray(edge_index[0], dtype=np.int64)
    dst = np.asarray(edge_index[1], dtype=np.int64)
    deg = (np.bincount(dst, minlength=N) + 1).astype(np.float64)
    dinv = 1.0 / np.sqrt(deg)
    xs = (x * dinv[:, None]).astype(np.float16)  # dinv_src folded into rows

    # per-core deduped (block, src) tables
    cores = []
    counts = []
    for c in range(n_cores):
        lo, hi = c * ns, (c + 1) * ns
        m = (dst >= lo) & (dst < hi)
        s_e = np.concatenate([src[m], np.arange(lo, hi)])
        d_e = np.concatenate([dst[m] - lo, np.arange(ns)])
        blk_e = d_e >> 7
        rel_e = (d_e & 127).astype(np.int64)
        uq, inv = np.unique(blk_e * N + s_e, return_inverse=True)
        ublk = uq // N
        usrc = uq % N
        c_k = np.bincount(ublk, minlength=nblk)
        slot_u = np.arange(len(uq)) - np.concatenate(
            [[0], np.cumsum(c_k)])[ublk]
        cores.append((blk_e, rel_e, inv, ublk, usrc, slot_u))
        counts.append(c_k)

    # rank alignment: each core processes its blocks sorted by slot count so
    # the SPMD-shared per-position batch count tracks the aligned quantiles,
    # not the worst core at each block id. perms[c][t] = core-c block at
    # position t; host un-permutes the output.
    perms = [np.argsort(-c_k, kind="stable") for c_k in counts]
    cnt_pos = np.stack([counts[c][perms[c]] for c in range(n_cores)])
    nb_of_blk = np.maximum(1, -(-cnt_pos.max(axis=0) // 128))  # per position
    plan = Plan(F, nblk, nb_of_blk)
    pref = plan.PREF

    import concourse.mybir as _mybir
    ohnp = _mybir.dt.np(_mybir.dt.float8e4)

    consts = {
        "wt": np.ascontiguousarray(
            np.asarray(W, np.float32).T).astype(np.float16),
        "bvec": np.asarray(b, np.float32).reshape(F, 1),
        "wreg": np.ascontiguousarray(
            np.asarray(w_reg, np.float32).T).astype(np.float16),
        "breg": np.asarray(b_reg, np.float32).reshape(1, 1),
    }

    # group tables (shared): runlen/rowbase per group of GRP blocks
    ngrp = -(-nblk // GRP)
    g_of_blk = np.arange(nblk) // GRP
    runlen = np.array([pref[min((g + 1) * GRP, nblk)] - pref[g * GRP]
                       for g in range(ngrp)])
    plan.RUNLEN = runlen

    plan.perms = perms
    for c in range(n_cores):
        blk_e, rel_e, inv, ublk, usrc, slot_u = cores[c]
        lo = c * ns
        posmap = np.empty(nblk, dtype=np.int64)
        posmap[perms[c]] = np.arange(nblk)

        # slot rows: row = 128*pref[g*GRP] + p*runlen_g + (pref[t]-pref[g*GRP]) + j
        upos = posmap[ublk]
        g_u = g_of_blk[upos]
        p_u = slot_u % 128
        j_u = slot_u // 128
        assert np.all(j_u < nb_of_blk[upos])
        row_u = (128 * pref[g_u * GRP] + p_u * runlen[g_u]
                 + (pref[upos] - pref[g_u * GRP]) + j_u)
        xg = np.zeros((128 * plan.TOTB, F), dtype=np.float16)
        xg[row_u] = xs[usrc]

        # scatter matrix with multiplicities
        slot_e = slot_u[inv]
        col_e = (pref[posmap[blk_e]] + slot_e // 128) * 128 + rel_e
        oh_u = np.zeros((128, plan.TOTB * 128), dtype=np.uint16)
        np.add.at(oh_u, (slot_e % 128, col_e), 1)
        oh = oh_u.astype(np.float32).astype(ohnp)

        dpad = np.ones(plan.NPAD, dtype=np.float16)
        dpad[:ns] = dinv[lo:lo + ns].astype(np.float16)
        dp = dpad.reshape(nblk, BLK)[perms[c]].reshape(1, plan.NPAD)

        plan.in_maps.append({
            "xg": xg,
            "oh": np.ascontiguousarray(oh),
            "dinvp": np.ascontiguousarray(dp),
            **consts,
        })
    return plan


# ---------------------------------------------------------------------------
def build_nc(plan):
    import concourse.bacc as bacc
    import concourse.mybir as mybir
    import concourse.tile as tile

    f32 = mybir.dt.float32
    f16 = mybir.dt.float16
    oh8 = mybir.dt.float8e4
    F, NBLK, NPAD, TOTB = plan.F, plan.NBLK, plan.NPAD, plan.TOTB
    NB, PREF, RUNLEN = plan.NB, plan.PREF, plan.RUNLEN
    NGRP = len(RUNLEN)
    RMAX = int(RUNLEN.max())

    nc = bacc.Bacc("TRN2", target_bir_lowering=False, debug=False)

    xg = nc.dram_tensor("xg", [128 * TOTB, F], f16, kind="ExternalInput").ap()
    oh = nc.dram_tensor("oh", [128, TOTB * 128], oh8,
                        kind="ExternalInput").ap()
    dinvp = nc.dram_tensor("dinvp", [1, NPAD], f16, kind="ExternalInput").ap()
    wt = nc.dram_tensor("wt", [F, F], f16, kind="ExternalInput").ap()
    bvec = nc.dram_tensor("bvec", [F, 1], f32, kind="ExternalInput").ap()
    wreg = nc.dram_tensor("wreg", [F, 1], f16, kind="ExternalInput").ap()
    breg = nc.dram_tensor("breg", [1, 1], f32, kind="ExternalInput").ap()
    out = nc.dram_tensor("out", [1, NPAD], f32, kind="ExternalOutput").ap()

    mult = mybir.AluOpType.mult
    bypass = mybir.AluOpType.bypass

    with tile.TileContext(nc) as tc:
        with (
            tc.tile_pool(name="const", bufs=1) as cpool,
            tc.tile_pool(name="stream", bufs=4) as spool,
            tc.tile_pool(name="ohp", bufs=4) as opool,
            tc.tile_pool(name="dbp", bufs=2) as dpool,
            tc.tile_pool(name="ps", bufs=3, space="PSUM") as pspool,
            tc.tile_pool(name="hq", bufs=3) as hqpool,
            tc.tile_pool(name="ph2", bufs=2, space="PSUM") as ph2pool,
            tc.tile_pool(name="po", bufs=2, space="PSUM") as popool,
            tc.tile_pool(name="hrelu", bufs=2) as hpool,
        ):
            wt_sb = cpool.tile([F, F], f16)
            b_sb = cpool.tile([F, 1], f32)
            wreg_sb = cpool.tile([F, 1], f16)
            breg_sb = cpool.tile([1, 1], f32)
            dinvp_sb = cpool.tile([1, NPAD], f16)
            out_sb = cpool.tile([1, NPAD], f32)

            def issue_group_dma(g):
                rl = int(RUNLEN[g])
                r0 = 128 * int(PREF[g * GRP])
                st = spool.tile([128, RMAX * F], f16, tag="st")
                nc.sync.dma_start(
                    out=st[:, :rl * F].rearrange("p (c f) -> p c f", f=F),
                    in_=xg[r0:r0 + 128 * rl, :].rearrange(
                        "(p c) f -> p c f", p=128),
                )
                ot = opool.tile([128, RMAX * BLK], oh8, tag="ot")
                # issue from sync too: the scalar engine's ACTIVATEs wait on
                # PE results and would head-of-line block these issues
                nc.sync.dma_start(
                    out=ot[:, :rl * BLK],
                    in_=oh[:, int(PREF[g * GRP]) * BLK:
                           (int(PREF[g * GRP]) + rl) * BLK])
                return st, ot

            # group 0's DMAs go first: descriptor-issue instructions cost
            # ~0.6-1.4us each on the issuing engine, so consts behind them
            # ride out group 0's transfer time instead of delaying it
            tiles0 = issue_group_dma(0)
            for sb, dr in ((dinvp_sb, dinvp), (wt_sb, wt), (b_sb, bvec),
                           (wreg_sb, wreg), (breg_sb, breg)):
                nc.sync.dma_start(out=sb[:], in_=dr[:])

            def phase2(hq, k0, cw):
                ph = ph2pool.tile([128, CH], f32)
                nc.tensor.matmul(ph[:, :cw], lhsT=wt_sb[:],
                                 rhs=hq[:, :cw], start=True, stop=True)
                hr = hpool.tile([128, CH], f16, tag="hr")
                nc.scalar.activation(hr[:, :cw], ph[:, :cw],
                                     mybir.ActivationFunctionType.Relu,
                                     bias=b_sb[:, :1])
                po = popool.tile([1, CH], f32)
                nc.tensor.matmul(po[:, :cw], lhsT=wreg_sb[:],
                                 rhs=hr[:, :cw], start=True, stop=True)
                nc.scalar.activation(out_sb[:, k0 * BLK:k0 * BLK + cw],
                                     po[:, :cw],
                                     mybir.ActivationFunctionType.Identity,
                                     bias=breg_sb[:, :1])
                # flush this group's output slice right away so only the
                # last slice remains for the kernel tail
                nc.scalar.dma_start(out=out[:, k0 * BLK:k0 * BLK + cw],
                                    in_=out_sb[:, k0 * BLK:k0 * BLK + cw])

            pending = None  # phase 2 lags one group so its matmuls never
            for g in range(NGRP):  # head-of-line block the PE FIFO
                k0 = g * GRP
                kn = min(GRP, NBLK - k0)
                cw = kn * BLK
                rl = int(RUNLEN[g])

                st, ot = tiles0 if g == 0 else issue_group_dma(g)

                db = dpool.tile([128, CH], f16, tag="db")
                nc.gpsimd.partition_broadcast(
                    db[:, :cw], dinvp_sb[:, k0 * BLK:k0 * BLK + cw])

                ps = pspool.tile([128, CH], f32)
                for bi in range(kn):
                    k = k0 + bi
                    nbk = int(NB[k])
                    base = int(PREF[k]) - int(PREF[k0])
                    for j in range(nbk):
                        cix = base + j
                        nc.tensor.matmul(
                            ps[:, bi * BLK:(bi + 1) * BLK],
                            lhsT=st[:, cix * F:(cix + 1) * F],
                            rhs=ot[:, cix * BLK:(cix + 1) * BLK],
                            start=(j == 0), stop=(j == nbk - 1))

                hq = hqpool.tile([128, CH], f16, tag="hq")
                nc.vector.scalar_tensor_tensor(
                    out=hq[:, :cw], in0=ps[:, :cw], scalar=1.0,
                    in1=db[:, :cw], op0=bypass, op1=mult)

                if pending is not None:
                    phase2(*pending)
                pending = (hq, k0, cw)
            phase2(*pending)

    nc.compile()
    return nc


# ---------------------------------------------------------------------------
_CACHE = {}


def _ensure_ntff_hook():
    try:
        from antenv.axon_hooks import get_axon_ntff_profile_hook  # noqa: F401
        return
    except ImportError:
        pass
    import sys
    import types
    import antenv
    mod = types.ModuleType("antenv.axon_hooks")
    mod._hook = None
    mod.set_axon_ntff_profile_hook = lambda h: setattr(mod, "_hook", h)
    mod.get_axon_ntff_profile_hook = lambda: mod._hook
    sys.modules["antenv.axon_hooks"] = mod
    antenv.axon_hooks = mod
    try:
        from trn_agent_boot.trn_boot import _ntff_profile_via_ctypes
        mod._hook = _ntff_profile_via_ctypes("/opt/axon/libaxon_pjrt.so")
    except Exception:
        pass


def _run(plan, nc, trace=False):
    import concourse.bass_utils as bu
    if trace:
        _ensure_ntff_hook()
        bu.upload_artifacts = lambda tmpdir: tmpdir  # no egress here
    core_ids = list(range(len(plan.in_maps)))
    res = bu.run_bass_kernel_spmd(nc, plan.in_maps, core_ids, trace=trace)
    return res


def kernel(x, edge_index, W, b, w_reg, b_reg):
    trace = bool(os.environ.get("GCN_TRACE"))

    plan = make_plan(x, edge_index, W, b, w_reg, b_reg)
    key = (plan.NBLK, plan.TOTB)
    if key not in _CACHE:
        _CACHE[key] = build_nc(plan)
    nc = _CACHE[key]

    res = None
    for attempt in range(3):
        try:
            res = _run(plan, nc, trace=trace)
            break
        except Exception:
            # transient device errors (e.g. NRT exec-unit resets) recover on
            # a fresh attempt; re-raise only if persistent
            if attempt == 2:
                raise
            time.sleep(5.0)
    kernel.last_exec_ns = res.exec_time_ns
    kernel.last_profile = res.profile_json

    N = np.asarray(x).shape[0]
    ns = N // len(plan.in_maps)
    shards = []
    for c in range(len(plan.in_maps)):
        o = res.results[c]["out"][0].reshape(plan.NBLK, BLK)
        unperm = np.empty_like(o)
        unperm[plan.perms[c]] = o  # position t holds block perms[c][t]
        shards.append(unperm.reshape(-1)[:ns])
    return np.concatenate(shards).reshape(N, 1).astype(np.float32)


kernel.last_exec_ns = None
kernel.last_profile = None


# revision 16
# speedup vs baseline: 1.0503x; 1.0249x over previous
"""GCN (single GCNConv + Cox head) Trainium2 Bass kernel, 8-core SPMD.

Math (per reference):
    src,dst += self loops;  deg = indegree(dst);  dinv = deg^-1/2
    agg[d]  = dinv[d] * sum_e 1[dst_e = d] * (dinv[src_e] * x[src_e])
    out     = relu(agg @ W.T + b) @ w_reg.T + b_reg

Distribution: destination-sharded over 8 cores (12500 dst nodes each), no
collectives — each core gets its own relabeled tables + scatter matrices
and writes its output shard; the host concatenates shards.

Per core the dst range is cut into 128-node blocks; blocks into groups of
GRP=4 (one [128, 512] PSUM bank per group). Per block, slots = the block's
DISTINCT sources (dinv[src] pre-folded into the stored fp16 rows); each
slot's scatter row carries the edge multiplicity at every dst it feeds
(counts are exact in fp8), so repeated sources need no extra slots and no
gather path at all. Slots are padded per block to NB_k*128 (NB_k shared
across cores so the SPMD instruction stream is identical).

Pipeline per group (all HWDGE-streamed, no SWDGE):
  - stream DMA: slot rows in matmul layout (slot i -> partition i%128)
  - scatter DMA: oh[slot, dst] fp8, one 128-col batch per 128 slots
  - PE: psum[f, dst] += msg[slot, f].T @ oh[slot, dst]  (fp16 x fp8)
  - DVE: accq = psum * dinv_dst (per-column, via gpsimd-broadcast row)
  - PE/ACT: h = relu(W.T @ accq + b); cox row = w_reg.T @ h + b_reg
  - one DMA writes the [1, 12544] output row at the end.
"""

import os
import time
import numpy as np

N_CORES = 8
BLK = 128      # dst nodes per block == scatter window
GRP = 4        # blocks per group == one [128, 512] psum bank
CH = GRP * BLK


class Plan:
    def __init__(self, n_feat, nblk, nb_of_blk):
        self.F = n_feat
        self.NBLK = nblk
        self.NB = nb_of_blk                      # batches per block [nblk]
        self.PREF = np.concatenate([[0], np.cumsum(nb_of_blk)])
        self.TOTB = int(self.PREF[-1])
        self.NPAD = nblk * BLK
        self.in_maps = []


def make_plan(x, edge_index, W, b, w_reg, b_reg, n_cores=N_CORES):
    x = np.asarray(x, dtype=np.float32)
    N, F = x.shape
    ns = N // n_cores
    assert ns * n_cores == N
    nblk = (ns + BLK - 1) // BLK

    src = np.asarray(edge_index[0], dtype=np.int64)
    dst = np.asarray(edge_index[1], dtype=np.int64)
    deg = (np.bincount(dst, minlength=N) + 1).astype(np.float64)
    dinv = 1.0 / np.sqrt(deg)
    xs = (x * dinv[:, None]).astype(np.float16)  # dinv_src folded into rows

    # per-core deduped (block, src) tables
    cores = []
    counts = []
    for c in range(n_cores):
        lo, hi = c * ns, (c + 1) * ns
        m = (dst >= lo) & (dst < hi)
        s_e = np.concatenate([src[m], np.arange(lo, hi)])
        d_e = np.concatenate([dst[m] - lo, np.arange(ns)])
        blk_e = d_e >> 7
        rel_e = (d_e & 127).astype(np.int64)
        uq, inv = np.unique(blk_e * N + s_e, return_inverse=True)
        ublk = uq // N
        usrc = uq % N
        c_k = np.bincount(ublk, minlength=nblk)
        slot_u = np.arange(len(uq)) - np.concatenate(
            [[0], np.cumsum(c_k)])[ublk]
        cores.append((blk_e, rel_e, inv, ublk, usrc, slot_u))
        counts.append(c_k)

    # rank alignment: each core processes its blocks sorted by slot count so
    # the SPMD-shared per-position batch count tracks the aligned quantiles,
    # not the worst core at each block id. perms[c][t] = core-c block at
    # position t; host un-permutes the output.
    perms = [np.argsort(-c_k, kind="stable") for c_k in counts]
    cnt_pos = np.stack([counts[c][perms[c]] for c in range(n_cores)])
    nb_of_blk = np.maximum(1, -(-cnt_pos.max(axis=0) // 128))  # per position
    plan = Plan(F, nblk, nb_of_blk)
    pref = plan.PREF

    import concourse.mybir as _mybir
    ohnp = _mybir.dt.np(_mybir.dt.float8e4)

    consts = {
        "wt": np.ascontiguousarray(
            np.asarray(W, np.float32).T).astype(np.float16),
        "bvec": np.asarray(b, np.float32).reshape(F, 1),
        "wreg": np.ascontiguousarray(
            np.asarray(w_reg, np.float32).T).astype(np.float16),
        "breg": np.asarray(b_reg, np.float32).reshape(1, 1),
    }

    # group tables (shared): runlen/rowbase per group of GRP blocks
    ngrp = -(-nblk // GRP)
    g_of_blk = np.arange(nblk) // GRP
    runlen = np.array([pref[min((g + 1) * GRP, nblk)] - pref[g * GRP]
                       for g in range(ngrp)])
    plan.RUNLEN = runlen

    plan.perms = perms
    for c in range(n_cores):
        blk_e, rel_e, inv, ublk, usrc, slot_u = cores[c]
        lo = c * ns
        posmap = np.empty(nblk, dtype=np.int64)
        posmap[perms[c]] = np.arange(nblk)

        # slot rows: row = 128*pref[g*GRP] + p*runlen_g + (pref[t]-pref[g*GRP]) + j
        upos = posmap[ublk]
        g_u = g_of_blk[upos]
        p_u = slot_u % 128
        j_u = slot_u // 128
        assert np.all(j_u < nb_of_blk[upos])
        row_u = (128 * pref[g_u * GRP] + p_u * runlen[g_u]
                 + (pref[upos] - pref[g_u * GRP]) + j_u)
        xg = np.zeros((128 * plan.TOTB, F), dtype=np.float16)
        xg[row_u] = xs[usrc]

        # scatter matrix with multiplicities
        slot_e = slot_u[inv]
        col_e = (pref[posmap[blk_e]] + slot_e // 128) * 128 + rel_e
        oh_u = np.zeros((128, plan.TOTB * 128), dtype=np.uint16)
        np.add.at(oh_u, (slot_e % 128, col_e), 1)
        oh = oh_u.astype(np.float32).astype(ohnp)

        dpad = np.ones(plan.NPAD, dtype=np.float16)
        dpad[:ns] = dinv[lo:lo + ns].astype(np.float16)
        dp = dpad.reshape(nblk, BLK)[perms[c]].reshape(1, plan.NPAD)

        plan.in_maps.append({
            "xg": xg,
            "oh": np.ascontiguousarray(oh),
            "dinvp": np.ascontiguousarray(dp),
            **consts,
        })
    return plan


# ---------------------------------------------------------------------------
def build_nc(plan):
    import concourse.bacc as bacc
    import concourse.mybir as mybir
    import concourse.tile as tile

    f32 = mybir.dt.float32
    f16 = mybir.dt.float16
    oh8 = mybir.dt.float8e4
    F, NBLK, NPAD, TOTB = plan.F, plan.NBLK, plan.NPAD, plan.TOTB
    NB, PREF, RUNLEN = plan.NB, plan.PREF, plan.RUNLEN
    NGRP = len(RUNLEN)
    RMAX = int(RUNLEN.max())

    nc = bacc.Bacc("TRN2", target_bir_lowering=False, debug=False)

    xg = nc.dram_tensor("xg", [128 * TOTB, F], f16, kind="ExternalInput").ap()
    oh = nc.dram_tensor("oh", [128, TOTB * 128], oh8,
                        kind="ExternalInput").ap()
    dinvp = nc.dram_tensor("dinvp", [1, NPAD], f16, kind="ExternalInput").ap()
    wt = nc.dram_tensor("wt", [F, F], f16, kind="ExternalInput").ap()
    bvec = nc.dram_tensor("bvec", [F, 1], f32, kind="ExternalInput").ap()
    wreg = nc.dram_tensor("wreg", [F, 1], f16, kind="ExternalInput").ap()
    breg = nc.dram_tensor("breg", [1, 1], f32, kind="ExternalInput").ap()
    out = nc.dram_tensor("out", [1, NPAD], f32, kind="ExternalOutput").ap()

    mult = mybir.AluOpType.mult
    bypass = mybir.AluOpType.bypass

    with tile.TileContext(nc) as tc:
        with (
            tc.tile_pool(name="const", bufs=1) as cpool,
            tc.tile_pool(name="stream", bufs=5) as spool,
            tc.tile_pool(name="ohp", bufs=5) as opool,
            tc.tile_pool(name="dbp", bufs=2) as dpool,
            tc.tile_pool(name="ps", bufs=4, space="PSUM") as pspool,
            tc.tile_pool(name="hq", bufs=3) as hqpool,
            tc.tile_pool(name="ph2", bufs=2, space="PSUM") as ph2pool,
            tc.tile_pool(name="po", bufs=2, space="PSUM") as popool,
            tc.tile_pool(name="hrelu", bufs=2) as hpool,
        ):
            wt_sb = cpool.tile([F, F], f16)
            b_sb = cpool.tile([F, 1], f32)
            wreg_sb = cpool.tile([F, 1], f16)
            breg_sb = cpool.tile([1, 1], f32)
            dinvp_sb = cpool.tile([1, NPAD], f16)
            out_sb = cpool.tile([1, NPAD], f32)

            def issue_group_dma(g):
                rl = int(RUNLEN[g])
                r0 = 128 * int(PREF[g * GRP])
                st = spool.tile([128, RMAX * F], f16, tag="st")
                nc.sync.dma_start(
                    out=st[:, :rl * F].rearrange("p (c f) -> p c f", f=F),
                    in_=xg[r0:r0 + 128 * rl, :].rearrange(
                        "(p c) f -> p c f", p=128),
                )
                ot = opool.tile([128, RMAX * BLK], oh8, tag="ot")
                # issue from sync too: the scalar engine's ACTIVATEs wait on
                # PE results and would head-of-line block these issues
                nc.sync.dma_start(
                    out=ot[:, :rl * BLK],
                    in_=oh[:, int(PREF[g * GRP]) * BLK:
                           (int(PREF[g * GRP]) + rl) * BLK])
                return st, ot

            # group 0's DMAs go first: descriptor-issue instructions cost
            # ~0.6-1.4us each on the issuing engine, so consts behind them
            # ride out group 0's transfer time instead of delaying it.
            # group 0 is further split per block (region-level deps) so its
            # first matmuls start after ~1/4 of the transfer.
            rl0 = int(RUNLEN[0])
            st0 = spool.tile([128, RMAX * F], f16, tag="st")
            ot0 = opool.tile([128, RMAX * BLK], oh8, tag="ot")
            xg0 = xg[0:128 * rl0, :].rearrange("(p c) f -> p c f", p=128)

            def issue_g0_block(bi):
                base = int(PREF[bi])
                nb = int(NB[bi])
                nc.sync.dma_start(
                    out=st0[:, base * F:(base + nb) * F].rearrange(
                        "p (c f) -> p c f", f=F),
                    in_=xg0[:, base:base + nb, :])
                nc.sync.dma_start(
                    out=ot0[:, base * BLK:(base + nb) * BLK],
                    in_=oh[:, base * BLK:(base + nb) * BLK])

            issue_g0_block(0)
            nc.sync.dma_start(out=dinvp_sb[:], in_=dinvp[:])
            for bi in range(1, min(GRP, NBLK)):
                issue_g0_block(bi)
            tiles0 = (st0, ot0)
            for sb, dr in ((wt_sb, wt), (b_sb, bvec),
                           (wreg_sb, wreg), (breg_sb, breg)):
                nc.sync.dma_start(out=sb[:], in_=dr[:])

            def phase2(hq, k0, cw):
                ph = ph2pool.tile([128, CH], f32)
                nc.tensor.matmul(ph[:, :cw], lhsT=wt_sb[:],
                                 rhs=hq[:, :cw], start=True, stop=True)
                hr = hpool.tile([128, CH], f16, tag="hr")
                nc.scalar.activation(hr[:, :cw], ph[:, :cw],
                                     mybir.ActivationFunctionType.Relu,
                                     bias=b_sb[:, :1])
                po = popool.tile([1, CH], f32)
                nc.tensor.matmul(po[:, :cw], lhsT=wreg_sb[:],
                                 rhs=hr[:, :cw], start=True, stop=True)
                nc.scalar.activation(out_sb[:, k0 * BLK:k0 * BLK + cw],
                                     po[:, :cw],
                                     mybir.ActivationFunctionType.Identity,
                                     bias=breg_sb[:, :1])
                # flush this group's output slice right away so only the
                # last slice remains for the kernel tail
                nc.scalar.dma_start(out=out[:, k0 * BLK:k0 * BLK + cw],
                                    in_=out_sb[:, k0 * BLK:k0 * BLK + cw])

            pending = None  # phase 2 lags one group so its matmuls never
            for g in range(NGRP):  # head-of-line block the PE FIFO
                k0 = g * GRP
                kn = min(GRP, NBLK - k0)
                cw = kn * BLK
                rl = int(RUNLEN[g])

                st, ot = tiles0 if g == 0 else issue_group_dma(g)

                db = dpool.tile([128, CH], f16, tag="db")
                nc.gpsimd.partition_broadcast(
                    db[:, :cw], dinvp_sb[:, k0 * BLK:k0 * BLK + cw])

                ps = pspool.tile([128, CH], f32)
                for bi in range(kn):
                    k = k0 + bi
                    nbk = int(NB[k])
                    base = int(PREF[k]) - int(PREF[k0])
                    for j in range(nbk):
                        cix = base + j
                        nc.tensor.matmul(
                            ps[:, bi * BLK:(bi + 1) * BLK],
                            lhsT=st[:, cix * F:(cix + 1) * F],
                            rhs=ot[:, cix * BLK:(cix + 1) * BLK],
                            start=(j == 0), stop=(j == nbk - 1))

                hq = hqpool.tile([128, CH], f16, tag="hq")
                nc.vector.scalar_tensor_tensor(
                    out=hq[:, :cw], in0=ps[:, :cw], scalar=1.0,
                    in1=db[:, :cw], op0=bypass, op1=mult)

                if pending is not None:
                    phase2(*pending)
                pending = (hq, k0, cw)
            phase2(*pending)

    nc.compile()
    return nc


# ---------------------------------------------------------------------------
_CACHE = {}


def _ensure_ntff_hook():
    try:
        from antenv.axon_hooks import get_axon_ntff_profile_hook  # noqa: F401
        return
    except ImportError:
        pass
    import sys
    import types
    import antenv
    mod = types.ModuleType("antenv.axon_hooks")
    mod._hook = None
    mod.set_axon_ntff_profile_hook = lambda h: setattr(mod, "_hook", h)
    mod.get_axon_ntff_profile_hook = lambda: mod._hook
    sys.modules["antenv.axon_hooks"] = mod
    antenv.axon_hooks = mod
    try:
        from trn_agent_boot.trn_boot import _ntff_profile_via_ctypes
        mod._hook = _ntff_profile_via_ctypes("/opt/axon/libaxon_pjrt.so")
    except Exception:
        pass


def _run(plan, nc, trace=False):
    import concourse.bass_utils as bu
    if trace:
        _ensure_ntff_hook()
        bu.upload_artifacts = lambda tmpdir: tmpdir  # no egress here
    core_ids = list(range(len(plan.in_maps)))
    res = bu.run_bass_kernel_spmd(nc, plan.in_maps, core_ids, trace=trace)
    return res


def kernel(x, edge_index, W, b, w_reg, b_reg):
    trace = bool(os.environ.get("GCN_TRACE"))

    plan = make_plan(x, edge_index, W, b, w_reg, b_reg)
    key = (plan.NBLK, plan.TOTB)
    if key not in _CACHE:
        _CACHE[key] = build_nc(plan)
    nc = _CACHE[key]

    res = None
    for attempt in range(3):
        try:
            res = _run(plan, nc, trace=trace)
            break
        except Exception:
            # transient device errors (e.g. NRT exec-unit resets) recover on
            # a fresh attempt; re-raise only if persistent
            if attempt == 2:
                raise
            time.sleep(5.0)
    kernel.last_exec_ns = res.exec_time_ns
    kernel.last_profile = res.profile_json

    N = np.asarray(x).shape[0]
    ns = N // len(plan.in_maps)
    shards = []
    for c in range(len(plan.in_maps)):
        o = res.results[c]["out"][0].reshape(plan.NBLK, BLK)
        unperm = np.empty_like(o)
        unperm[plan.perms[c]] = o  # position t holds block perms[c][t]
        shards.append(unperm.reshape(-1)[:ns])
    return np.concatenate(shards).reshape(N, 1).astype(np.float32)


kernel.last_exec_ns = None
kernel.last_profile = None
